# revision 23
# baseline (speedup 1.0000x reference)
"""Expert-parallel MoE kernel for Trainium2 (8 NeuronCores).

Problem: top-2 MoE, N=8192 tokens, D=1024, H=4096, E=8 experts.
Strategy (expert parallel):
  - Host: compute gating (logits -> top-k -> softmax) exactly as the
    reference does (CPU jax, fp32), dispatch tokens to their experts.
  - Core e holds expert e's weights; it runs a 2-layer MLP over the
    tokens routed to it (padded to a fixed capacity C), plus the
    combine() row-renormalization:
        y = (relu(x @ w1 + b1) @ w2 + b2)
        y_scaled = y * (gate * ||x||) / (||y|| + 1e-8)
  - Host: scatter-add per-expert outputs back to the [N, D] result.

Device kernel (per core, bf16 matmuls, fp32 PSUM accumulation):
  Token blocks of <=512. Layer 1 computes hT [H, R] (H on partitions) by
  streaming w1 in per-h-tile chunks; layer 2 accumulates out[R, D] in
  PSUM over the 32 H-tiles with w2 resident in SBUF. Epilogue: +b2,
  row sum-of-squares (ACT Square with accum_out), sqrt, reciprocal,
  final scale, DMA out.

  Inputs are pre-tiled on the host so every DMA chunk is contiguous per
  partition (2-8KB runs; untiled layouts measured only ~138GB/s):
    xT  [P, n_k*C]        xT[p, n_k*B + k*R + j] = x[tok B+j, k*128+p]
    w1  [P, n_h, n_k, P]  w1[p, h, k, j] = w1[k*128+p, h*128+j]
    w2  [P, n_h, D]       w2[p, h, d]    = w2[h*128+p, d]
  DMA queue discipline: x/w1 stream on the sync-engine HWDGE queue; the
  8MB w2 load on the scalar-engine queue (delayed behind the first
  stream chunk); y outputs on the gpsimd SWDGE queue (an engine-FIFO
  DMA trigger on ACT would block layer-1 relu evacuation).
"""

import os
import sys

import numpy as np

if "/opt/trn_rl_repo" not in sys.path:
    sys.path.insert(0, "/opt/trn_rl_repo")

import ml_dtypes

N, D, H, E = 8192, 1024, 4096, 8
P = 128
BLK = 512  # max token block
NK = D // P   # 8
NH = H // P   # 32
BF16 = ml_dtypes.bfloat16

_nc_cache = {}


def _blocks_for(C):
    # Full blocks first, small remainder last: a leading small block would
    # make layer 1 consume w1 at ~580GB/s (N=128 matmuls) and stall on HBM;
    # as the last block its layer 1 prefetches under the previous block's
    # layer 2 instead.
    blocks = []
    off = 0
    while off < C:
        r = min(BLK, C - off)
        blocks.append((off, r))
        off += r
    return blocks


def _tile_w1(w1e):
    """[D, H] fp32 -> [P, NH, NK, P] bf16 with w1t[p,h,k,j] = w1e[k*P+p, h*P+j]."""
    return np.ascontiguousarray(
        w1e.reshape(NK, P, NH, P).transpose(1, 2, 0, 3).astype(BF16))


def _tile_w2(w2e):
    """[H, D] fp32 -> [P, NH, D] bf16 with w2t[p,h,d] = w2e[h*P+p, d]."""
    return np.ascontiguousarray(
        w2e.reshape(NH, P, D).transpose(1, 0, 2).astype(BF16))


def _tile_xT(xg, C):
    """[C, D] fp32 (padded) -> [P, NK*C] bf16, per-block [k, j] segments."""
    out = np.zeros((P, NK * C), BF16)
    for B, R in _blocks_for(C):
        seg = xg[B:B + R].T.reshape(NK, P, R).transpose(1, 0, 2)
        out[:, NK * B:NK * (B + R)] = seg.reshape(P, NK * R)
    return out


def _build_nc(C):
    """Build the per-core Bass program for capacity C (multiple of 128)."""
    from contextlib import ExitStack

    import concourse.bass as bass
    import concourse.mybir as mybir
    import concourse.tile as tile
    from concourse import bacc

    f32 = mybir.dt.float32
    bf16 = mybir.dt.bfloat16
    AF = mybir.ActivationFunctionType

    nc = bacc.Bacc(trn_type="TRN2", num_devices=E)
    xT = nc.dram_tensor("xT", [P, NK * C], bf16, kind="ExternalInput")
    w1 = nc.dram_tensor("w1", [P, NH, NK, P], bf16, kind="ExternalInput")
    b1 = nc.dram_tensor("b1", [P, NH], f32, kind="ExternalInput")
    w2 = nc.dram_tensor("w2", [P, NH, D], bf16, kind="ExternalInput")
    b2 = nc.dram_tensor("b2", [D], f32, kind="ExternalInput")
    sc = nc.dram_tensor("sc", [P, C // P], f32, kind="ExternalInput")
    y = nc.dram_tensor("y", [C, D], bf16, kind="ExternalOutput")

    y_t = y.ap().rearrange("(o p) d -> p o d", p=P)

    blocks = _blocks_for(C)

    with tile.TileContext(nc) as tc, ExitStack() as ctx:
        singles = ctx.enter_context(tc.tile_pool(name="singles", bufs=1))
        xpool = ctx.enter_context(tc.tile_pool(name="xpool", bufs=2))
        w1pool = ctx.enter_context(tc.tile_pool(name="w1pool", bufs=8))
        hpool = ctx.enter_context(tc.tile_pool(name="hpool", bufs=2))
        stpool = ctx.enter_context(tc.tile_pool(name="stpool", bufs=2))
        sqpool = ctx.enter_context(tc.tile_pool(name="sqpool", bufs=2))
        smpool = ctx.enter_context(tc.tile_pool(name="smpool", bufs=4))
        ybpool = ctx.enter_context(tc.tile_pool(name="ybpool", bufs=3))
        psh = ctx.enter_context(tc.tile_pool(name="psh", bufs=4, space="PSUM"))
        pso = ctx.enter_context(tc.tile_pool(name="pso", bufs=2, space="PSUM"))

        # --- preamble: constants ---
        b1_sb = singles.tile([P, NH], f32)
        nc.gpsimd.dma_start(out=b1_sb, in_=b1.ap())
        b2_sb = singles.tile([P, D], f32)
        b2_bcast = bass.AP(tensor=b2.ap().tensor, offset=b2.ap().offset,
                           ap=[[0, P], *b2.ap().ap])
        nc.gpsimd.dma_start(out=b2_sb, in_=b2_bcast)
        sc_sb = singles.tile([P, C // P], f32)
        nc.gpsimd.dma_start(out=sc_sb, in_=sc.ap())
        # w2 is loaded in 1MB chunks spread through block-0's layer 1 (the
        # triggers sit between relus in the ACT FIFO), so it neither hogs
        # HBM during startup nor misses its first layer-2 use.
        w2_sb = singles.tile([P, NH, D], bf16)

        for (B, R) in blocks:
            m_tiles = R // P
            xt = xpool.tile([P, NK, BLK], bf16, tag="xt", name="xt")[:, :, :R]
            xsrc = xT.ap()[:, NK * B:NK * (B + R)].rearrange(
                "p (k j) -> p k j", k=NK)
            if B == 0:
                # Split so the h=0/k=0 matmul waits on 128KB, not 1MB.
                nc.sync.dma_start(out=xt[:, :1, :], in_=xsrc[:, :1, :])
                nc.sync.dma_start(out=xt[:, 1:, :], in_=xsrc[:, 1:, :])
            else:
                nc.sync.dma_start(out=xt, in_=xsrc)

            # --- layer 1: hT[h, tok] = relu(x @ w1 + b1), H on partitions ---
            hT = hpool.tile([P, NH, BLK], bf16, tag="hT", name="hT")[:, :, :R]
            for h in range(NH):
                w1c = w1pool.tile([P, NK, P], bf16, tag="w1c")
                if B == 0 and h == 0:
                    nc.sync.dma_start(out=w1c[:, :1, :], in_=w1.ap()[:, h, :1, :])
                    nc.sync.dma_start(out=w1c[:, 1:, :], in_=w1.ap()[:, h, 1:, :])
                else:
                    nc.sync.dma_start(out=w1c, in_=w1.ap()[:, h])
                if B == 0 and h % 4 == 3:
                    # w2 rows ride the same FIFO queue, paced between the
                    # w1 chunks so they never starve the layer-1 stream.
                    nc.sync.dma_start(out=w2_sb[:, h - 3:h + 1, :],
                                      in_=w2.ap()[:, h - 3:h + 1, :])
                ps = psh.tile([P, BLK], f32, tag="ph", name="ph")[:, :R]
                for k in range(NK):
                    nc.tensor.matmul(
                        ps,
                        lhsT=w1c[:, k, :],
                        rhs=xt[:, k, :],
                        start=(k == 0),
                        stop=(k == NK - 1),
                    )
                nc.scalar.activation(
                    out=hT[:, h, :], in_=ps, func=AF.Relu,
                    bias=b1_sb[:, h:h + 1], scale=1.0,
                )

            # --- layer 2: out[tok, D] accumulated over h; eager epilogue ---
            for m in range(m_tiles):
                po = pso.tile([P, D], f32, tag="po")
                for h in range(NH):
                    for n2 in range(2):
                        nc.tensor.matmul(
                            po[:, n2 * 512:(n2 + 1) * 512],
                            lhsT=hT[:, h, m * P:(m + 1) * P],
                            rhs=w2_sb[:, h, n2 * 512:(n2 + 1) * 512],
                            start=(h == 0),
                            stop=(h == NH - 1),
                        )
                # stage = out + b2 ; q = sum(stage^2) ; f = sc/(sqrt(q)+eps)
                is_final = (B, R) == blocks[-1] and m == m_tiles - 1
                stage = stpool.tile([P, D], f32, tag="stage", name="stage")
                if is_final:
                    # Kernel tail: pipeline add/square in D-halves so the
                    # exposed chain after the last matmul shrinks.
                    qa = smpool.tile([P, 1], f32, tag="q", name="qa")
                    qb = smpool.tile([P, 1], f32, tag="q", name="qb")
                    sqh = sqpool.tile([P, D], f32, tag="sq")
                    for half, qh in ((0, qa), (1, qb)):
                        sl = slice(half * 512, half * 512 + 512)
                        nc.vector.tensor_add(out=stage[:, sl], in0=po[:, sl],
                                             in1=b2_sb[:, sl])
                        nc.scalar.activation(
                            out=sqh[:, sl], in_=stage[:, sl], func=AF.Square,
                            accum_out=qh,
                        )
                    q = smpool.tile([P, 1], f32, tag="q", name="q")
                    nc.vector.tensor_add(out=q, in0=qa, in1=qb)
                else:
                    nc.vector.tensor_add(out=stage, in0=po, in1=b2_sb)
                    sq = sqpool.tile([P, D], f32, tag="sq")
                    q = smpool.tile([P, 1], f32, tag="q", name="q")
                    nc.scalar.activation(
                        out=sq, in_=stage, func=AF.Square, accum_out=q,
                    )
                f = smpool.tile([P, 1], f32, tag="f", name="f")
                nc.scalar.activation(out=f, in_=q, func=AF.Sqrt)
                nc.vector.tensor_scalar_add(out=f, in0=f, scalar1=1e-8)
                nc.vector.reciprocal(out=f, in_=f)
                nc.vector.tensor_mul(out=f, in0=f,
                                     in1=sc_sb[:, B // P + m:B // P + m + 1])
                yb = ybpool.tile([P, D], bf16, tag="yb", name="yb")
                nc.vector.tensor_scalar_mul(out=yb, in0=stage, scalar1=f)
                nc.gpsimd.dma_start(out=y_t[:, B // P + m, :], in_=yb)

    nc.compile()
    return nc


def _get_nc(C):
    if C not in _nc_cache:
        _nc_cache[C] = _build_nc(C)
    return _nc_cache[C]


LAST_EXEC_NS = None
LAST_TRACE = None


def _install_axon_ntff_hook():
    """Register antenv.axon_hooks shim driving NTFF capture via the axon .so.

    The agent image's antenv package lacks axon_hooks, so concourse's
    trace=True path degrades. Replicates trn_boot._ntff_profile_via_ctypes.
    """
    import contextlib
    import ctypes
    import types

    if "antenv.axon_hooks" in sys.modules:
        return
    lib = ctypes.CDLL("/opt/axon/libaxon_pjrt.so")
    if not hasattr(lib, "axon_start_nrt_profile"):
        return
    lib.axon_start_nrt_profile.argtypes = [ctypes.POINTER(ctypes.c_int64),
                                           ctypes.c_size_t]
    lib.axon_start_nrt_profile.restype = ctypes.c_int64
    lib.axon_stop_nrt_profile.argtypes = [ctypes.c_char_p]
    lib.axon_stop_nrt_profile.restype = ctypes.c_int64

    @contextlib.contextmanager
    def _hook(output_dir, device_ids):
        import jax
        jax.devices()
        if device_ids:
            ids = (ctypes.c_int64 * len(device_ids))(*device_ids)
            rc = lib.axon_start_nrt_profile(ids, len(device_ids))
        else:
            rc = lib.axon_start_nrt_profile(None, 0)
        if rc != 0:
            raise RuntimeError(f"axon_start_nrt_profile rc={rc}")
        try:
            yield
        finally:
            n = lib.axon_stop_nrt_profile(str(output_dir).encode())
            print(f"ntff capture: {n} file(s) -> {output_dir}", file=sys.stderr)

    mod = types.ModuleType("antenv.axon_hooks")
    mod.get_axon_ntff_profile_hook = lambda: _hook
    sys.modules["antenv.axon_hooks"] = mod
    import antenv
    antenv.axon_hooks = mod


def _gating(x, w_gate, k):
    """Top-k gating computed exactly like the reference (CPU jax, fp32)."""
    import jax
    import jax.numpy as jnp

    cpu = jax.devices("cpu")[0]
    with jax.default_device(cpu):
        xj = jnp.asarray(x)
        logits = xj @ jnp.asarray(w_gate)
        top_vals, top_idx = jax.lax.top_k(logits, k)
        top_gates = jax.nn.softmax(top_vals, axis=-1)
        init_norm = jnp.linalg.norm(xj, axis=-1)
        return (np.asarray(top_idx), np.asarray(top_gates, np.float32),
                np.asarray(init_norm, np.float32))


def kernel(x, w_gate, w1, b1, w2, b2, k):
    from concourse.bass_utils import run_bass_kernel_spmd

    x = np.asarray(x, np.float32)
    w_gate = np.asarray(w_gate, np.float32)
    w1 = np.asarray(w1, np.float32)
    b1 = np.asarray(b1, np.float32)
    w2 = np.asarray(w2, np.float32)
    b2 = np.asarray(b2, np.float32)
    k = int(np.asarray(k))
    n, d = x.shape
    e = w_gate.shape[1]

    top_idx, top_gates, init_norm = _gating(x, w_gate, k)

    idxs, scs = [], []
    for ei in range(e):
        tok, slot = np.nonzero(top_idx == ei)
        idxs.append(tok)
        scs.append(top_gates[tok, slot] * init_norm[tok])

    maxc = max(len(t) for t in idxs)
    C = max(((maxc + P - 1) // P) * P, P)
    nc = _get_nc(C)

    in_maps = []
    for ei in range(e):
        tok = idxs[ei]
        xg = np.zeros((C, d), np.float32)
        xg[:len(tok)] = x[tok]
        sce = np.zeros((C,), np.float32)
        sce[:len(tok)] = scs[ei]
        sce = np.ascontiguousarray(sce.reshape(C // P, P).T)
        in_maps.append({
            "xT": _tile_xT(xg, C),
            "w1": _tile_w1(w1[ei]),
            "b1": np.ascontiguousarray(b1[ei].reshape(NH, P).T),
            "w2": _tile_w2(w2[ei]),
            "b2": np.ascontiguousarray(b2[ei]),
            "sc": sce,
        })

    trace = bool(int(os.environ.get("MOE_TRACE", "0")))
    kwargs = {}
    if trace:
        _install_axon_ntff_hook()
        tdir = os.environ.get("MOE_TRACE_DIR")
        if tdir:
            os.makedirs(tdir, exist_ok=True)
            kwargs["tmpdir"] = tdir
        kwargs["trace_cores"] = [0]
    res = run_bass_kernel_spmd(
        nc, in_maps, core_ids=list(range(e)), trace=trace, **kwargs,
    )
    global LAST_EXEC_NS, LAST_TRACE
    LAST_EXEC_NS = res.exec_time_ns
    LAST_TRACE = res.instructions_and_trace
    if res.exec_time_ns is not None:
        print(f"HW exec time: {res.exec_time_ns} ns", file=sys.stderr)

    y = np.zeros((n, d), np.float32)
    for ei in range(e):
        tok = idxs[ei]
        y[tok] += res.results[ei]["y"][:len(tok)].astype(np.float32)
    return y

